# revision 11
# baseline (speedup 1.0000x reference)
"""DeepseekMoE kernel for 8 Trainium2 NeuronCores.

Strategy (expert-parallel routed + data-parallel shared, fp8 DoubleRow):
  - Host computes the router (gate matmul, softmax, top-2) in numpy and
    gathers each expert's tokens (MoE dispatch as part of sharding).
  - Core c runs routed expert c's FFN over its gathered tokens; shared
    experts are replicated and each core runs them over a distinct
    512-token slice of the batch.
  - All heavy matmuls use fp8(e4m3) in DoubleRow perf mode: each
    instruction contracts 2 k-tiles (256 rows) at 0.5 cycles per output
    column -- 4x the fp16 rate on the PE.
  - Accuracy: weights are pre-scaled by 2^11 so their hi/lo fp8 splits
    stay out of e4m3's subnormal floor; the descale folds into the ACT
    input scale (layer 1) and the host-side combine weights (layer 2).
      * shared experts (~98% of output norm): layer 1 runs a 3-matmul
        hi/lo compensation (w_hi*x_hi + w_lo*x_hi + w_hi*x_lo), h is
        kept in fp16 and layer 2 runs as a plain fp16 matmul.
      * routed experts (~21% of output norm): plain fp8 both layers.
    Measured end-to-end rel err ~1.1% vs the 2e-2 gate.
  - Layer 2 keeps tokens on the output partition dim (stationary = h,
    moving = w2), so each 128-token block accumulates into exactly one
    PSUM bank, the combine weight is a per-partition tensor_scalar
    multiply, and outputs land in [token, D] layout (no host transpose).
  - Routed and shared f-tile steps are interleaved (Bresenham weave) so
    the ACT-heavy routed phase overlaps the PE-heavy shared phase, with
    a 2-step software pipeline between layer 1 and layer 2.
  - 256-token chunks keep both phases' PSUM footprint at 8 banks total
    (2x p1 double-buffer + 2 token-block accumulators per phase).
"""

import numpy as np
import ml_dtypes

import concourse.bass as bass
import concourse.tile as tile
import concourse.mybir as mybir
from concourse import bacc
from concourse.bass_utils import run_bass_kernel_spmd

B, S, D, F, E, NS, K = 2, 2048, 512, 2048, 8, 2, 2
T = B * S
N_CORES = 8
TS = T // N_CORES          # shared-expert tokens per core
FS = NS * F                # concatenated shared FFN width
CS = 256                   # token chunk (1 PSUM bank per 2-f-tile p1 pair)
KD = D // 128              # 4 k-tiles over D
FR = F // 128              # 16 f-tiles routed
FSH = FS // 128            # 32 f-tiles shared
PR = FR // 2               # 8 routed f-pairs
PS = FSH // 2              # 16 shared f-pairs
WS = 2048.0                # 2^11 weight pre-scale (exact power of two)

F8 = mybir.dt.float8e4
F16 = mybir.dt.float16
F32 = mybir.dt.float32
np8 = ml_dtypes.float8_e4m3
np16 = np.float16

_GELU = mybir.ActivationFunctionType.Gelu
_DR = mybir.MatmulPerfMode.DoubleRow

_cache: dict = {}


def _chunks(total):
    """(start, size) chunk list: CS-sized chunks plus a ragged tail."""
    out, c0 = [], 0
    while c0 < total:
        cs = min(CS, total - c0)
        out.append((c0, cs))
        c0 += cs
    return out


def _weave(ra, sb):
    """Evenly interleave two lists (Bresenham pacing)."""
    out, i, j = [], 0, 0
    while i < len(ra) or j < len(sb):
        if j >= len(sb) or (i < len(ra) and i * len(sb) <= j * len(ra)):
            out.append(ra[i]); i += 1
        else:
            out.append(sb[j]); j += 1
    return out


def _build(cpad: int):
    nc = bacc.Bacc("TRN2", debug=False)

    rchunks = _chunks(cpad)
    schunks = _chunks(TS)
    ntb_r = sum(-(-cs // 128) for _, cs in rchunks)

    xg = nc.dram_tensor("xg", [128, KD * cpad], F8, kind="ExternalInput")
    cwp = nc.dram_tensor("cwp", [128, ntb_r], F32, kind="ExternalInput")
    rw1 = nc.dram_tensor("rw1", [128, KD * F], F8, kind="ExternalInput")
    rw2 = nc.dram_tensor("rw2", [128, F * KD], F8, kind="ExternalInput")
    xsh = nc.dram_tensor("xsh", [128, KD * TS], F8, kind="ExternalInput")
    xsl = nc.dram_tensor("xsl", [128, KD * TS], F8, kind="ExternalInput")
    sw1h = nc.dram_tensor("sw1h", [128, KD * FS], F8, kind="ExternalInput")
    sw1l = nc.dram_tensor("sw1l", [128, KD * FS], F8, kind="ExternalInput")
    sw2 = nc.dram_tensor("sw2", [128, FSH * D], F16, kind="ExternalInput")
    yr = nc.dram_tensor("yr", [cpad, D], F16, kind="ExternalOutput")
    ys = nc.dram_tensor("ys", [TS, D], F16, kind="ExternalOutput")

    with tile.TileContext(nc) as tc:
        with (
            tc.tile_pool(name="wts", bufs=1) as wts,
            tc.tile_pool(name="acts", bufs=1) as acts,
            tc.tile_pool(name="hpr", bufs=4) as hpr,
            tc.tile_pool(name="hps", bufs=4) as hps,
            tc.tile_pool(name="op", bufs=3) as op,
            tc.tile_pool(name="psr", bufs=2, space="PSUM") as psr,
            tc.tile_pool(name="pss", bufs=2, space="PSUM") as pss,
            tc.tile_pool(name="por", bufs=1, space="PSUM") as por,
            tc.tile_pool(name="pos", bufs=1, space="PSUM") as pos,
        ):
            # ---- warmup while the first DMAs are in flight: trigger the
            # GELU table load and ramp the PE p-state ----
            warm = wts.tile([128, 256], F8, name="warm_in")
            nc.vector.memset(warm[:], 0.0)
            wh = hpr.tile([128, 256], F8, name="warm_h")
            nc.scalar.activation(wh[:], warm[:, 0:256], _GELU, bias=0.0)
            wp = psr.tile([128, 512], F32, tag="p1", name="warm_p")
            w3 = warm.rearrange("p (two n) -> p two n", two=2)
            for _ in range(8):
                nc.tensor.matmul(wp[:, 0:128], w3[:, :, 0:128], w3[:],
                                 start=True, stop=True, perf_mode=_DR)

            # ---- resident SBUF images ----
            xg_sb = acts.tile([128, KD * cpad], F8, name="xg_sb")
            cwp_sb = acts.tile([128, ntb_r], F32, name="cwp_sb")
            rw1_sb = wts.tile([128, KD * F], F8, name="rw1_sb")
            rw2_sb = wts.tile([128, F * KD], F8, name="rw2_sb")
            xsh_sb = acts.tile([128, KD * TS], F8, name="xsh_sb")
            xsl_sb = acts.tile([128, KD * TS], F8, name="xsl_sb")
            sw1h_sb = wts.tile([128, KD * FS], F8, name="sw1h_sb")
            sw1l_sb = wts.tile([128, KD * FS], F8, name="sw1l_sb")
            sw2_sb = wts.tile([128, FSH * D], F16, name="sw2_sb")

            # 3-d views for DoubleRow operand slicing
            rw1_v = rw1_sb.rearrange("p (k f) -> p k f", k=KD)
            rw2_v = rw2_sb.rearrange("p (j two d) -> p j two d", j=PR, two=2)
            sw1h_v = sw1h_sb.rearrange("p (k f) -> p k f", k=KD)
            sw1l_v = sw1l_sb.rearrange("p (k f) -> p k f", k=KD)
            sw2_v = sw2_sb.rearrange("p (f d) -> p f d", f=FSH)
            xg_v = xg_sb  # chunk-major; sliced via offsets below
            xoff = [0]
            for _, cs in rchunks:
                xoff.append(xoff[-1] + KD * cs)
            soff = [0]
            for _, cs in schunks:
                soff.append(soff[-1] + KD * cs)

            def w1_group_dma(dst, src, f_lo, f_hi, n_f):
                d3 = dst.rearrange("p (k f) -> p k f", k=KD)
                s3 = src.ap().rearrange("p (k f) -> p k f", k=KD)
                nc.sync.dma_start(d3[:, :, f_lo * 128:f_hi * 128],
                                  s3[:, :, f_lo * 128:f_hi * 128])

            # ---- consumption-ordered preload: small first groups so the
            # first R and S pair-steps unblock ASAP, then stream the rest ----
            w1_group_dma(rw1_sb, rw1, 0, 2, FR)
            nc.gpsimd.dma_start(xg_sb[:, 0:xoff[1]], xg.ap()[:, 0:xoff[1]])
            nc.sync.dma_start(rw2_sb[:, 0:2 * 2 * D], rw2.ap()[:, 0:2 * 2 * D])
            w1_group_dma(sw1h_sb, sw1h, 0, 2, FSH)
            w1_group_dma(sw1l_sb, sw1l, 0, 2, FSH)
            nc.gpsimd.dma_start(xsh_sb[:], xsh.ap())
            nc.gpsimd.dma_start(xsl_sb[:], xsl.ap())
            w1_group_dma(rw1_sb, rw1, 2, 4, FR)
            nc.sync.dma_start(sw2_sb[:, 0:2 * D], sw2.ap()[:, 0:2 * D])
            w1_group_dma(sw1h_sb, sw1h, 2, 6, FSH)
            w1_group_dma(sw1l_sb, sw1l, 2, 6, FSH)
            nc.sync.dma_start(sw2_sb[:, 2 * D:4 * D], sw2.ap()[:, 2 * D:4 * D])
            w1_group_dma(rw1_sb, rw1, 4, 8, FR)
            nc.sync.dma_start(cwp_sb[:], cwp.ap())
            w1_group_dma(sw1h_sb, sw1h, 6, 10, FSH)
            w1_group_dma(sw1l_sb, sw1l, 6, 10, FSH)
            nc.sync.dma_start(sw2_sb[:, 4 * D:8 * D], sw2.ap()[:, 4 * D:8 * D])
            nc.sync.dma_start(rw2_sb[:, 2 * 2 * D:5 * 2 * D],
                              rw2.ap()[:, 2 * 2 * D:5 * 2 * D])
            if len(rchunks) > 1:
                nc.gpsimd.dma_start(xg_sb[:, xoff[1]:xoff[2]],
                                    xg.ap()[:, xoff[1]:xoff[2]])
            w1_group_dma(rw1_sb, rw1, 8, 12, FR)
            w1_group_dma(sw1h_sb, sw1h, 10, 16, FSH)
            w1_group_dma(sw1l_sb, sw1l, 10, 16, FSH)
            nc.sync.dma_start(sw2_sb[:, 8 * D:14 * D], sw2.ap()[:, 8 * D:14 * D])
            nc.sync.dma_start(rw2_sb[:, 5 * 2 * D:], rw2.ap()[:, 5 * 2 * D:])
            if len(rchunks) > 2:
                nc.gpsimd.dma_start(xg_sb[:, xoff[2]:xoff[3]],
                                    xg.ap()[:, xoff[2]:xoff[3]])
            w1_group_dma(rw1_sb, rw1, 12, 16, FR)
            w1_group_dma(sw1h_sb, sw1h, 16, 22, FSH)
            w1_group_dma(sw1l_sb, sw1l, 16, 22, FSH)
            nc.sync.dma_start(sw2_sb[:, 14 * D:20 * D], sw2.ap()[:, 14 * D:20 * D])
            if len(rchunks) > 3:
                nc.gpsimd.dma_start(xg_sb[:, xoff[3]:], xg.ap()[:, xoff[3]:])
            w1_group_dma(sw1h_sb, sw1h, 22, 28, FSH)
            w1_group_dma(sw1l_sb, sw1l, 22, 28, FSH)
            nc.sync.dma_start(sw2_sb[:, 20 * D:26 * D], sw2.ap()[:, 20 * D:26 * D])
            w1_group_dma(sw1h_sb, sw1h, 28, 32, FSH)
            w1_group_dma(sw1l_sb, sw1l, 28, 32, FSH)
            nc.sync.dma_start(sw2_sb[:, 26 * D:], sw2.ap()[:, 26 * D:])

            # ---- step list: (phase, chunk_idx, pair_idx), woven ----
            rsteps = [("R", ci, j) for ci in range(len(rchunks))
                      for j in range(PR)]
            ssteps = [("S", ci, j) for ci in range(len(schunks))
                      for j in range(PS)]
            # R pairs are ready first (small x/w1 groups); give them a head
            # start so the PE has work while the shared weights stream in
            head = min(4, len(rsteps))
            steps = rsteps[:head] + _weave(rsteps[head:], ssteps)

            # per-chunk global t-block column base for cwp
            tb_base = []
            acc = 0
            for _, cs in rchunks:
                tb_base.append(acc)
                acc += -(-cs // 128)

            h_tiles: dict = {}
            po_r: list = [None]
            po_s: list = [None]

            def stage_a(step):
                ph, ci, j = step
                if ph == "R":
                    c0, cs = rchunks[ci]
                    p1 = psr.tile([128, 2 * cs], F32, tag="p1", name="p1r")
                    for i01 in range(2):
                        f = 2 * j + i01
                        dst = p1[:, i01 * cs:(i01 + 1) * cs]
                        xo = xoff[ci]
                        for b in range(2):
                            nc.tensor.matmul(
                                dst,
                                rw1_v[:, 2 * b:2 * b + 2, f * 128:(f + 1) * 128],
                                xg_v[:, xo:xo + KD * cs].rearrange(
                                    "p (k c) -> p k c", k=KD)[:, 2 * b:2 * b + 2, :],
                                start=(b == 0), stop=(b == 1), perf_mode=_DR)
                    h = hpr.tile([128, 2 * cs], F8, name="hr")
                    nc.scalar.activation(h[:], p1[:], _GELU, bias=0.0,
                                         scale=1.0 / WS)
                else:
                    c0, cs = schunks[ci]
                    p1 = pss.tile([128, 2 * cs], F32, tag="p1", name="p1s")
                    xh3 = xsh_sb[:, soff[ci]:soff[ci] + KD * cs].rearrange(
                        "p (k c) -> p k c", k=KD)
                    xl3 = xsl_sb[:, soff[ci]:soff[ci] + KD * cs].rearrange(
                        "p (k c) -> p k c", k=KD)
                    for i01 in range(2):
                        f = 2 * j + i01
                        dst = p1[:, i01 * cs:(i01 + 1) * cs]
                        fs = slice(f * 128, (f + 1) * 128)
                        for b in range(2):
                            ks = slice(2 * b, 2 * b + 2)
                            nc.tensor.matmul(
                                dst, sw1h_v[:, ks, fs], xh3[:, ks, :],
                                start=(b == 0), stop=False, perf_mode=_DR)
                            nc.tensor.matmul(
                                dst, sw1l_v[:, ks, fs], xh3[:, ks, :],
                                start=False, stop=False, perf_mode=_DR)
                            nc.tensor.matmul(
                                dst, sw1h_v[:, ks, fs], xl3[:, ks, :],
                                start=False, stop=(b == 1), perf_mode=_DR)
                    h = hps.tile([128, 2 * cs], F16, name="hs")
                    nc.scalar.activation(h[:], p1[:], _GELU, bias=0.0,
                                         scale=1.0 / WS)
                return h

            def stage_b(step, h, is_last_step):
                ph, ci, j = step
                if ph == "R":
                    c0, cs = rchunks[ci]
                    ntb = -(-cs // 128)
                    if j == 0:
                        po_r[0] = [por.tile([128, D], F32, tag=f"r{tb}",
                                            name=f"por{tb}")
                                   for tb in range(ntb)]
                    h3 = h.rearrange("p (two c) -> p two c", two=2)
                    for tb in range(ntb):
                        tbs = min(128, cs - tb * 128)
                        nc.tensor.matmul(
                            po_r[0][tb][0:tbs, :],
                            h3[:, :, tb * 128:tb * 128 + tbs],
                            rw2_v[:, j, :, :],
                            start=(j == 0), stop=(j == PR - 1), perf_mode=_DR)
                    if j == PR - 1:
                        # per-t-block evac + DMA so the store of tb0 overlaps
                        # the evac of tb1 (pipelined drain)
                        o = op.tile([128, ntb * D], F16, name="or")
                        for tb in range(ntb):
                            tbs = min(128, cs - tb * 128)
                            nc.vector.tensor_scalar_mul(
                                o[0:tbs, tb * D:(tb + 1) * D],
                                po_r[0][tb][0:tbs, :],
                                cwp_sb[0:tbs, tb_base[ci] + tb:
                                       tb_base[ci] + tb + 1])
                            nc.gpsimd.dma_start(
                                yr.ap()[c0 + tb * 128:c0 + tb * 128 + tbs, :],
                                o[0:tbs, tb * D:(tb + 1) * D])
                else:
                    c0, cs = schunks[ci]
                    ntb = -(-cs // 128)
                    if j == 0:
                        po_s[0] = [pos.tile([128, D], F32, tag=f"s{tb}",
                                            name=f"pos{tb}")
                                   for tb in range(ntb)]
                    for i01 in range(2):
                        f = 2 * j + i01
                        for tb in range(ntb):
                            tbs = min(128, cs - tb * 128)
                            nc.tensor.matmul(
                                po_s[0][tb][0:tbs, :],
                                h[:, i01 * cs + tb * 128:
                                  i01 * cs + tb * 128 + tbs],
                                sw2_v[:, f, :],
                                start=(f == 0), stop=(f == FSH - 1))
                    if j == PS - 1:
                        # evacs split ACT/DVE so the two t-blocks drain in
                        # parallel, each followed by its own store
                        o = op.tile([128, ntb * D], F16, name="os")
                        for tb in range(ntb):
                            tbs = min(128, cs - tb * 128)
                            if tb % 2 == 1:
                                nc.scalar.copy(o[0:tbs, tb * D:(tb + 1) * D],
                                               po_s[0][tb][0:tbs, :])
                            else:
                                nc.vector.tensor_copy(
                                    o[0:tbs, tb * D:(tb + 1) * D],
                                    po_s[0][tb][0:tbs, :])
                            nc.gpsimd.dma_start(
                                ys.ap()[c0 + tb * 128:c0 + tb * 128 + tbs, :],
                                o[0:tbs, tb * D:(tb + 1) * D])

            LOOKAHEAD = 3
            for i in range(len(steps) + LOOKAHEAD):
                if i < len(steps):
                    h_tiles[i] = stage_a(steps[i])
                jj = i - LOOKAHEAD
                if jj >= 0:
                    stage_b(steps[jj], h_tiles.pop(jj), jj == len(steps) - 1)

    nc.compile()
    return nc


def _pack_k_blocks(a2d, dtype):
    """[K*128, N] -> [128, K*N] with k-blocks along the free dim."""
    k = a2d.shape[0] // 128
    return np.ascontiguousarray(
        a2d.reshape(k, 128, -1).transpose(1, 0, 2).reshape(128, -1)
        .astype(dtype))


def _pack_chunked(xT, total, dtype):
    """[D, total] -> [128, KD*total] chunk-major k-blocks."""
    parts = []
    for c0, cs in _chunks(total):
        blk = xT[:, c0:c0 + cs]
        parts.append(blk.reshape(KD, 128, cs).transpose(1, 0, 2)
                     .reshape(128, -1))
    return np.ascontiguousarray(np.concatenate(parts, axis=1).astype(dtype))


def _split8(a):
    """Scaled hi/lo e4m3 split of an array (applied at scale WS)."""
    hi = a.astype(np8)
    lo = (a - hi.astype(np.float32)).astype(np8)
    return hi, lo


def _numpy_fallback(x, gate_w, gate_b, sw1, sb1, sw2, sb2, rw1, rb1, rw2, rb2):
    from scipy.special import erf
    t = x.reshape(-1, D)
    gelu = lambda u: 0.5 * u * (1.0 + erf(u / np.sqrt(2.0)))
    hs = gelu(np.einsum('td,nfd->ntf', t, sw1) + sb1[:, None, :])
    shared = np.einsum('ntf,ndf->td', hs, sw2) + sb2.sum(axis=0)
    logits = t @ gate_w.T + gate_b
    m = logits.max(axis=1, keepdims=True)
    ex = np.exp(logits - m)
    probs = ex / ex.sum(axis=1, keepdims=True)
    top_i = np.argpartition(-probs, K - 1, axis=1)[:, :K]
    cw = np.zeros_like(probs)
    np.add.at(cw, (np.arange(t.shape[0])[:, None], top_i),
              np.take_along_axis(probs, top_i, axis=1))
    hr = gelu(np.einsum('td,efd->etf', t, rw1) + rb1[:, None, :])
    oe = np.einsum('etf,edf->etd', hr, rw2) + rb2[:, None, :]
    routed = np.einsum('etd,te->td', oe, cw)
    return (shared + routed).reshape(x.shape).astype(np.float32)


def kernel(x, gate_w, gate_b, sw1, sb1, sw2, sb2, rw1, rb1, rw2, rb2):
    x = np.asarray(x, np.float32)
    gate_w = np.asarray(gate_w, np.float32)
    gate_b = np.asarray(gate_b, np.float32)
    sw1 = np.asarray(sw1, np.float32)
    sb1 = np.asarray(sb1, np.float32)
    sw2 = np.asarray(sw2, np.float32)
    sb2 = np.asarray(sb2, np.float32)
    rw1 = np.asarray(rw1, np.float32)
    rb1 = np.asarray(rb1, np.float32)
    rw2 = np.asarray(rw2, np.float32)
    rb2 = np.asarray(rb2, np.float32)

    if sb1.any() or rb1.any():
        # device path folds first-layer biases away (they are zero in the
        # problem spec); fall back to exact numpy if that ever changes
        return _numpy_fallback(x, gate_w, gate_b, sw1, sb1, sw2, sb2,
                               rw1, rb1, rw2, rb2)

    t = x.reshape(T, D)

    # ---- router on host (dispatch/sharding step) ----
    logits = t @ gate_w.T + gate_b
    m = logits.max(axis=1, keepdims=True)
    ex = np.exp(logits - m)
    probs = ex / ex.sum(axis=1, keepdims=True)
    top_i = np.argpartition(-probs, K - 1, axis=1)[:, :K]

    sel = np.zeros((T, E), bool)
    sel[np.arange(T)[:, None], top_i] = True
    idxs = [np.nonzero(sel[:, e])[0] for e in range(E)]
    counts = np.array([len(i) for i in idxs])
    # multiple of 128 so every chunk splits into full 128-token t-blocks
    # (DoubleRow ldweights rejects partial stationary tiles)
    cpad = max(CS, int(-(-counts.max() // 128) * 128))

    if cpad not in _cache:
        _cache[cpad] = _build(cpad)
    nc = _cache[cpad]

    ntb_r = sum(-(-cs // 128) for _, cs in _chunks(cpad))

    # ---- shared-expert packing (replicated) ----
    sw1s = sw1.reshape(FS, D).T * WS                      # [D, FS]
    s1h, s1l = _split8(sw1s)
    sw1h_p = _pack_k_blocks(s1h, np8)
    sw1l_p = _pack_k_blocks(s1l, np8)
    sw2t = sw2.transpose(0, 2, 1).reshape(FS, D)          # [FS, D]
    sw2_p = _pack_k_blocks(sw2t, np16)

    in_maps = []
    for c in range(N_CORES):
        idx = idxs[c]
        ce = len(idx)
        # routed tokens, fp8, chunk-major
        xgT = np.zeros((D, cpad), np.float32)
        xgT[:, :ce] = t[idx].T
        xg_p = _pack_chunked(xgT.astype(np8), cpad, np8)
        # combine weights / WS as per-partition scalars, t-block cols
        cw_col = np.zeros(ntb_r * 128, np.float32)
        cw_col[:ce] = probs[idx, c] / WS
        # account for chunk-major t-block layout: blocks follow token order
        cwp = np.zeros((128, ntb_r), np.float32)
        col = 0
        pos = 0
        for c0, cs in _chunks(cpad):
            ntb = -(-cs // 128)
            for tb in range(ntb):
                tbs = min(128, cs - tb * 128)
                cwp[:tbs, col] = cw_col[pos:pos + tbs]
                pos += tbs
                col += 1
        # routed weights: hi-only, scaled
        r1h = (rw1[c].T * WS).astype(np8)                 # [D, F]
        rw1_p = _pack_k_blocks(r1h, np8)
        r2h = (rw2[c].T * WS).astype(np8)                 # [F, D] (w2T)
        rw2_p = np.ascontiguousarray(
            r2h.reshape(PR, 2, 128, D).transpose(2, 0, 1, 3)
            .reshape(128, -1))
        # shared tokens hi/lo
        xsT = t[c * TS:(c + 1) * TS].T                    # [D, TS]
        xh = xsT.astype(np8)
        xl = (xsT - xh.astype(np.float32)).astype(np8)
        in_maps.append({
            "xg": xg_p,
            "cwp": cwp,
            "rw1": rw1_p,
            "rw2": rw2_p,
            "xsh": _pack_chunked(xh, TS, np8),
            "xsl": _pack_chunked(xl, TS, np8),
            "sw1h": sw1h_p,
            "sw1l": sw1l_p,
            "sw2": sw2_p,
        })

    res = run_bass_kernel_spmd(nc, in_maps, core_ids=list(range(N_CORES)))

    # ---- combine on host ----
    out = np.empty((T, D), np.float32)
    for c in range(N_CORES):
        out[c * TS:(c + 1) * TS] = res.results[c]["ys"].astype(np.float32)
    for c in range(N_CORES):
        idx = idxs[c]
        out[idx] += res.results[c]["yr"][:len(idx)].astype(np.float32)

    # output biases (zero in the spec, handled exactly anyway)
    if sb2.any() or rb2.any():
        cw = np.zeros((T, E), np.float32)
        np.add.at(cw, (np.arange(T)[:, None], top_i),
                  np.take_along_axis(probs, top_i, axis=1))
        out += sb2.sum(axis=0)[None, :] + cw @ rb2

    return out.reshape(B, S, D)
